# revision 8
# baseline (speedup 1.0000x reference)
"""GQA attention kernel for 8 Trainium2 NeuronCores — v2.

Sharding: core c = 4*b + h handles batch b (of 2) and kv-head h (of 4),
i.e. one kv head + its 4 grouped query heads. Each core computes its head
group's partial contribution to the output projection; the host sums the
4 partials per batch. No collectives.

v2 changes vs v1 (461us):
- all matmul inputs bf16 (x, Wq/k/v already Wo) — halves DMA traffic;
  bf16 matmul is 1 cyc/row like f32r, but ldweights are cheaper
- softmax denominator: accumulate the exp tiles into R via two parallel
  chains (GpSimd: even sk-pairs, DVE: odd sk-pairs), fold, then ONE
  512-row matmul with an all-ones [128,128] stationary per (qtile, head)
  — the PE both reduces over partitions AND replicates den across all
  128 output partitions, so no gpsimd broadcast is needed. Replaces the
  per-sk ones-matmuls that were 1/3 of attention-phase PE rows.
- reciprocal_approx_fast instead of reciprocal (~5x faster)
- software pipelining: outproj(qt-1) i-blocks interleaved between
  attn(qt) head-groups — pure-PE outproj stretches let the scalar
  engine (exp is 1.11us/sk-pair vs PE's 0.85us) catch up; scores
  emitted one sk-pair ahead of AV; normalize deferred one head-group
- BAND 256->512: ap-512 projection matmuls

Device math per core (S=2048, H=2048, d=128):
  QT_g = (x @ Wq_g + bq_g)^T          [d, S]   g=0..3   (bf16)
  KT   = (x @ Wk_h)^T                 [d, S]            (bk cancels in softmax)
  V    = x @ Wv_h                     [S, d]   (computed as V^T then PE-transposed)
  S^T  = KT^T-blocks @ QT             [Sk, Sq]
  P^T  = exp(SCALE * S^T)             (bf16, no max-subtraction: |s| <~ 6)
  den  = ones.T @ (DVE-sum of P^T tiles)
  y^T  = V^T-blocks.T @ P^T (PSUM accum);  yT := y^T * (1/den)
  out += yT_g^T @ Wo_g                [S, H]  (partial over this head group)
Host: out[b] = sum_h partial + (bv_rep @ Wo + bo).
"""

import numpy as np
import ml_dtypes

B = 2
S = 2048
HIDDEN = 2048
NKV = 4
GROUP = 4
D = 128
SCALE = D ** -0.5

BAND = 512            # S-columns per projection band
NBAND = S // BAND     # 4
NCH = HIDDEN // 128   # 16 contraction chunks
QTILE = 512           # queries per attention tile
NQT = S // QTILE      # 4
NSK = S // 128        # 16 key tiles
NSKP = NSK // 2       # 8 sk pairs

_CACHE = {}
LAST_RESULTS = None
TRACE = False
TMPDIR = None


def _build():
    import concourse.bass as bass
    import concourse.bacc as bacc
    import concourse.mybir as mybir
    import concourse.tile as tile
    from concourse.masks import make_identity

    f32 = mybir.dt.float32
    bf16 = mybir.dt.bfloat16
    EXP = mybir.ActivationFunctionType.Exp
    IDENT = mybir.ActivationFunctionType.Identity
    COPY = mybir.ActivationFunctionType.Copy

    nc = bacc.Bacc(trn_type="TRN2", target_bir_lowering=False, debug=False)

    xT = nc.dram_tensor("xT", [NBAND, 128, NCH, BAND], bf16, kind="ExternalInput").ap()
    wq = nc.dram_tensor("wq", [GROUP, 128, NCH, 128], bf16, kind="ExternalInput").ap()
    wk = nc.dram_tensor("wk", [128, NCH, 128], bf16, kind="ExternalInput").ap()
    wv = nc.dram_tensor("wv", [128, NCH, 128], bf16, kind="ExternalInput").ap()
    wo = nc.dram_tensor("wo", [GROUP, 128, HIDDEN], bf16, kind="ExternalInput").ap()
    bq = nc.dram_tensor("bq", [128, GROUP], f32, kind="ExternalInput").ap()
    onesk = nc.dram_tensor("onesk", [128, 128], bf16, kind="ExternalInput").ap()
    out = nc.dram_tensor("out", [S, HIDDEN], f32, kind="ExternalOutput").ap()

    with tile.TileContext(nc) as tc:
        with (
            tc.tile_pool(name="const", bufs=1) as constp,
            tc.tile_pool(name="wts", bufs=1) as wtsp,
            tc.tile_pool(name="xb", bufs=2) as xbp,
            tc.tile_pool(name="qkv", bufs=1) as qkvp,
            tc.tile_pool(name="ptbuf", bufs=2) as ptp,
            tc.tile_pool(name="rbuf", bufs=2) as rp,
            tc.tile_pool(name="dens", bufs=2) as densp,
            tc.tile_pool(name="ytbuf", bufs=8) as ytp,
            tc.tile_pool(name="outbuf", bufs=2) as outp,
        ):
            # ---- DMAs in consumption order ----
            onesk_t = constp.tile([128, 128], bf16, name="onesk_t")
            nc.sync.dma_start(out=onesk_t[:, :], in_=onesk)
            bq_t = constp.tile([128, GROUP], f32, name="bq_t")
            nc.sync.dma_start(out=bq_t[:, :], in_=bq)
            ident = constp.tile([128, 128], f32, name="ident")
            make_identity(nc, ident[:, :])

            wk_t = wtsp.tile([128, NCH, 128], bf16, name="wk_t")
            nc.sync.dma_start(out=wk_t[:, :, :], in_=wk)

            # band 0 issued right after wk; remaining weights before band 1
            bands = [None] * NBAND
            bands[0] = xbp.tile([128, NCH, BAND], bf16, name="band", tag="band")
            nc.sync.dma_start(out=bands[0][:, :, :], in_=xT[0])

            wv_t = wtsp.tile([128, NCH, 128], bf16, name="wv_t")
            nc.sync.dma_start(out=wv_t[:, :, :], in_=wv)
            wq_t = []
            for g in range(GROUP):
                t = wtsp.tile([128, NCH, 128], bf16, name=f"wq_t{g}", tag=f"wq{g}")
                nc.sync.dma_start(out=t[:, :, :], in_=wq[g])
                wq_t.append(t)
            bands[1] = xbp.tile([128, NCH, BAND], bf16, name="band", tag="band")
            nc.sync.dma_start(out=bands[1][:, :, :], in_=xT[1])
            wo_t = []
            for g in range(GROUP):
                t = wtsp.tile([128, HIDDEN], bf16, name=f"wo_t{g}", tag=f"wo{g}")
                nc.sync.dma_start(out=t[:, :], in_=wo[g])
                wo_t.append(t)

            # ---- persistent activations ----
            qt_t = []
            for g in range(GROUP):
                t = qkvp.tile([128, S], bf16, name=f"qt{g}", tag=f"qt{g}")
                qt_t.append(t)
            kt_t = qkvp.tile([128, S], bf16, name="kt_t")
            v_t = qkvp.tile([128, S], bf16, name="v_t")
            vt_f = qkvp.tile([128, S], f32, name="vt_f")

            # =============== phase 1: projections ===============
            with tc.tile_pool(name="psA", bufs=1, space="PSUM") as psA:
                for bd in range(NBAND):
                    if bands[bd] is None:
                        bands[bd] = xbp.tile(
                            [128, NCH, BAND], bf16, name="band", tag="band"
                        )
                        nc.sync.dma_start(out=bands[bd][:, :, :], in_=xT[bd])
                    band = bands[bd]
                    bsl = slice(bd * BAND, (bd + 1) * BAND)

                    # K^T accumulation
                    pk = psA.tile([128, BAND], f32, name="pk", tag="pacc", bufs=3)
                    for c in range(NCH):
                        nc.tensor.matmul(
                            out=pk[:, :],
                            lhsT=wk_t[:, c, :],
                            rhs=band[:, c, :],
                            start=(c == 0), stop=(c == NCH - 1),
                        )
                    nc.scalar.activation(kt_t[:, bsl], pk[:, :], COPY)

                    # V^T accumulation (f32, transposed to V per 128-block later)
                    pv = psA.tile([128, BAND], f32, name="pv", tag="pacc", bufs=3)
                    for c in range(NCH):
                        nc.tensor.matmul(
                            out=pv[:, :],
                            lhsT=wv_t[:, c, :],
                            rhs=band[:, c, :],
                            start=(c == 0), stop=(c == NCH - 1),
                        )
                    nc.scalar.activation(vt_f[:, bsl], pv[:, :], COPY)

                    # Q^T per local head
                    for g in range(GROUP):
                        pq = psA.tile([128, BAND], f32, name="pq", tag="pacc", bufs=3)
                        for c in range(NCH):
                            nc.tensor.matmul(
                                out=pq[:, :],
                                lhsT=wq_t[g][:, c, :],
                                rhs=band[:, c, :],
                                start=(c == 0), stop=(c == NCH - 1),
                            )
                        nc.scalar.activation(
                            qt_t[g][:, bsl], pq[:, :], IDENT,
                            bias=bq_t[:, g:g + 1],
                        )

                    # transpose V^T band -> V (BAND//128 sk-tiles per band)
                    for t in range(BAND // 128):
                        sk = bd * (BAND // 128) + t
                        pt = psA.tile([128, 128], f32, name="ptr", tag="pacc", bufs=3)
                        nc.tensor.transpose(
                            pt[:, :], vt_f[:, sk * 128:(sk + 1) * 128], ident[:, :]
                        )
                        nc.scalar.activation(
                            v_t[:, sk * 128:(sk + 1) * 128], pt[:, :], COPY
                        )

            # =============== phase 2+3: attention + out-projection ===============
            with tc.tile_pool(name="psB", bufs=1, space="PSUM") as psB:
                yt_all = {}
                pending = [None]  # deferred normalize closure

                def flush_pending():
                    if pending[0] is not None:
                        pending[0]()
                        pending[0] = None

                def attn_group(qt, g):
                    qsl = slice(qt * QTILE, (qt + 1) * QTILE)
                    py = psB.tile([128, QTILE], f32, name="py", tag="yacc", bufs=2)
                    pt_all = ptp.tile(
                        [128, NSKP, 2 * QTILE], bf16, name="pt", tag="pt"
                    )
                    # two parallel accumulation chains: even sk-pairs on
                    # GpSimd, odd on DVE; merged + folded in normalize()
                    Re = rp.tile([128, 2 * QTILE], f32, name="re", tag="re")
                    Ro = rp.tile([128, 2 * QTILE], f32, name="ro", tag="ro")
                    for skp in range(NSKP):
                        ps = psB.tile(
                            [128, 2 * QTILE], f32, name="ps", tag="sc", bufs=2
                        )
                        for half in range(2):
                            sk = 2 * skp + half
                            nc.tensor.matmul(
                                out=ps[:, half * QTILE:(half + 1) * QTILE],
                                lhsT=kt_t[:, sk * 128:(sk + 1) * 128],
                                rhs=qt_t[g][:, qsl],
                                start=True, stop=True,
                            )
                        nc.scalar.activation(
                            pt_all[:, skp, :], ps[:, :], EXP, scale=SCALE
                        )
                        if skp == 2:
                            nc.gpsimd.tensor_add(
                                Re[:, :], pt_all[:, 0, :], pt_all[:, 2, :]
                            )
                        elif skp >= 4 and skp % 2 == 0:
                            nc.gpsimd.tensor_add(
                                Re[:, :], Re[:, :], pt_all[:, skp, :]
                            )
                        elif skp == 3:
                            nc.vector.tensor_add(
                                Ro[:, :], pt_all[:, 1, :], pt_all[:, 3, :]
                            )
                        elif skp >= 5 and skp % 2 == 1:
                            nc.vector.tensor_add(
                                Ro[:, :], Ro[:, :], pt_all[:, skp, :]
                            )
                        if skp >= 1:
                            # AV for the previous sk pair (scores stay one
                            # pair ahead so exp latency is hidden)
                            pskp = skp - 1
                            for half in range(2):
                                sk = 2 * pskp + half
                                nc.tensor.matmul(
                                    out=py[:, :],
                                    lhsT=v_t[:, sk * 128:(sk + 1) * 128],
                                    rhs=pt_all[:, pskp,
                                               half * QTILE:(half + 1) * QTILE],
                                    start=(sk == 0), stop=False,
                                )
                        if skp == 1:
                            flush_pending()
                    for half in range(2):
                        sk = 2 * (NSKP - 1) + half
                        nc.tensor.matmul(
                            out=py[:, :],
                            lhsT=v_t[:, sk * 128:(sk + 1) * 128],
                            rhs=pt_all[:, NSKP - 1,
                                       half * QTILE:(half + 1) * QTILE],
                            start=False, stop=(sk == NSK - 1),
                        )

                    def normalize(qt=qt, g=g, py=py, Re=Re, Ro=Ro):
                        nc.gpsimd.tensor_add(Re[:, :], Re[:, :], Ro[:, :])
                        rh = rp.tile([128, QTILE], bf16, name="rh", tag="rh")
                        nc.vector.tensor_add(
                            rh[:, :], Re[:, 0:QTILE], Re[:, QTILE:2 * QTILE]
                        )
                        # all-ones stationary: out[i, q] = sum_p rh[p, q] —
                        # partition-reduces AND replicates den on all 128
                        # partitions in one 512-row matmul
                        pden = psB.tile(
                            [128, QTILE], f32, name="pden", tag="den", bufs=2
                        )
                        nc.tensor.matmul(
                            out=pden[:, :],
                            lhsT=onesk_t[:, :],
                            rhs=rh[:, :],
                            start=True, stop=True,
                        )
                        rb = densp.tile([128, QTILE], f32, name="rb", tag="rb")
                        nc.vector.reciprocal_approx_fast(rb[:, :], pden[:, :])
                        yt = ytp.tile([128, QTILE], bf16, name="yt", tag="yt")
                        nc.vector.tensor_mul(yt[:, :], py[:, :], rb[:, :])
                        yt_all[(qt, g)] = yt

                    pending[0] = normalize

                def outproj_block(qt, i):
                    outs = outp.tile([128, HIDDEN], f32, name="outs", tag="outs")
                    po = [
                        psB.tile([128, 2 * QTILE], f32, name=f"po{jp}",
                                 tag="sc", bufs=2)
                        for jp in range(2)
                    ]
                    for g in range(GROUP):
                        lhs = yt_all[(qt, g)][:, i * 128:(i + 1) * 128]
                        for j in range(4):
                            nc.tensor.matmul(
                                out=po[j // 2][:, (j % 2) * 512:(j % 2 + 1) * 512],
                                lhsT=lhs,
                                rhs=wo_t[g][:, j * 512:(j + 1) * 512],
                                start=(g == 0), stop=(g == GROUP - 1),
                            )
                    nc.vector.tensor_copy(outs[:, 0:1024], po[0][:, :])
                    nc.vector.tensor_copy(outs[:, 1024:2048], po[1][:, :])
                    r0 = qt * QTILE + i * 128
                    nc.sync.dma_start(out=out[r0:r0 + 128, :], in_=outs[:, :])

                for qt in range(NQT):
                    for g in range(GROUP):
                        attn_group(qt, g)
                        if qt >= 1:
                            outproj_block(qt - 1, g)
                flush_pending()
                for i in range(QTILE // 128):
                    outproj_block(NQT - 1, i)

    nc.finalize()
    return nc


def _get_nc():
    if "nc" not in _CACHE:
        _CACHE["nc"] = _build()
    return _CACHE["nc"]


def kernel(x, Wq, bq, Wk, bk, Wv, bv, Wo, bo):
    global LAST_RESULTS
    from concourse.bass_utils import run_bass_kernel_spmd

    bf = ml_dtypes.bfloat16
    x = np.asarray(x, np.float32)
    Wq = np.asarray(Wq, np.float32)
    Wk = np.asarray(Wk, np.float32)
    Wv = np.asarray(Wv, np.float32)
    Wo = np.asarray(Wo, np.float32)
    bq = np.asarray(bq, np.float32)
    bv = np.asarray(bv, np.float32)
    bo = np.asarray(bo, np.float32)

    nc = _get_nc()

    onesk_np = np.ones((128, 128), bf)

    in_maps = []
    for c in range(8):
        b, h = divmod(c, NKV)
        xTb = x[b].T  # [HIDDEN, S]
        xTh = np.ascontiguousarray(
            xTb.reshape(NCH, 128, NBAND, BAND).transpose(2, 1, 0, 3)
        ).astype(bf)
        # wq[g]: [128, NCH, 128] per local head
        wqh = np.ascontiguousarray(
            Wq[:, h * 512:(h + 1) * 512]
            .reshape(NCH, 128, GROUP, 128).transpose(2, 1, 0, 3)
        ).astype(bf)
        wkh = np.ascontiguousarray(
            Wk[:, h * 128:(h + 1) * 128].reshape(NCH, 128, 128).transpose(1, 0, 2)
        ).astype(bf)
        wvh = np.ascontiguousarray(
            Wv[:, h * 128:(h + 1) * 128].reshape(NCH, 128, 128).transpose(1, 0, 2)
        ).astype(bf)
        woh = np.ascontiguousarray(
            Wo[h * 512:(h + 1) * 512, :].reshape(GROUP, 128, HIDDEN)
        ).astype(bf)
        bqh = np.ascontiguousarray(
            bq[h * 512:(h + 1) * 512].reshape(GROUP, 128).T
        )
        in_maps.append({
            "xT": xTh, "wq": wqh, "wk": wkh, "wv": wvh, "wo": woh,
            "bq": bqh, "onesk": onesk_np,
        })

    res = run_bass_kernel_spmd(
        nc, in_maps, list(range(8)), trace=TRACE, tmpdir=TMPDIR
    )
    LAST_RESULTS = res

    # host-side constant bias: (bv repeated per head group) @ Wo + bo
    bv_rep = np.broadcast_to(
        bv.reshape(NKV, 1, D), (NKV, GROUP, D)
    ).reshape(HIDDEN)
    bias_row = bv_rep @ Wo + bo  # [HIDDEN]

    out = np.empty((B, S, HIDDEN), np.float32)
    for b in range(B):
        acc = res.results[b * NKV + 0]["out"].astype(np.float32)
        for h in range(1, NKV):
            acc = acc + res.results[b * NKV + h]["out"]
        out[b] = acc + bias_row
    return out


# revision 12
# speedup vs baseline: 1.2651x; 1.2651x over previous
"""GQA attention kernel for 8 Trainium2 NeuronCores — v2.

Sharding: core c = 4*b + h handles batch b (of 2) and kv-head h (of 4),
i.e. one kv head + its 4 grouped query heads. Each core computes its head
group's partial contribution to the output projection; the host sums the
4 partials per batch. No collectives.

v2 changes vs v1 (461us):
- all matmul inputs bf16 (x, Wq/k/v already Wo) — halves DMA traffic;
  bf16 matmul is 1 cyc/row like f32r, but ldweights are cheaper
- softmax denominator: accumulate the exp tiles into R via two parallel
  chains (GpSimd: even sk-pairs, DVE: odd sk-pairs), fold, then ONE
  512-row matmul with an all-ones [128,128] stationary per (qtile, head)
  — the PE both reduces over partitions AND replicates den across all
  128 output partitions, so no gpsimd broadcast is needed. Replaces the
  per-sk ones-matmuls that were 1/3 of attention-phase PE rows.
- reciprocal_approx_fast instead of reciprocal (~5x faster)
- software pipelining: outproj(qt-1) i-blocks interleaved between
  attn(qt) head-groups — pure-PE outproj stretches let the scalar
  engine (exp is 1.11us/sk-pair vs PE's 0.85us) catch up; scores
  emitted one sk-pair ahead of AV; normalize deferred one head-group
- BAND 256->512: ap-512 projection matmuls

Device math per core (S=2048, H=2048, d=128):
  QT_g = (x @ Wq_g + bq_g)^T          [d, S]   g=0..3   (bf16)
  KT   = (x @ Wk_h)^T                 [d, S]            (bk cancels in softmax)
  V    = x @ Wv_h                     [S, d]   (computed as V^T then PE-transposed)
  S^T  = KT^T-blocks @ QT             [Sk, Sq]
  P^T  = exp(SCALE * S^T)             (bf16, no max-subtraction: |s| <~ 6)
  den  = ones.T @ (DVE-sum of P^T tiles)
  y^T  = V^T-blocks.T @ P^T (PSUM accum);  yT := y^T * (1/den)
  out += yT_g^T @ Wo_g                [S, H]  (partial over this head group)
Host: out[b] = sum_h partial + (bv_rep @ Wo + bo).
"""

import numpy as np
import ml_dtypes

B = 2
S = 2048
HIDDEN = 2048
NKV = 4
GROUP = 4
D = 128
SCALE = D ** -0.5

BAND = 512            # S-columns per projection band
NBAND = S // BAND     # 4
NCH = HIDDEN // 128   # 16 contraction chunks
QTILE = 512           # queries per attention tile
NQT = S // QTILE      # 4
NSK = S // 128        # 16 key tiles
NSKP = NSK // 2       # 8 sk pairs

_CACHE = {}
LAST_RESULTS = None
TRACE = False
TMPDIR = None


def _build():
    import concourse.bass as bass
    import concourse.bacc as bacc
    import concourse.mybir as mybir
    import concourse.tile as tile
    from concourse.masks import make_identity

    f32 = mybir.dt.float32
    bf16 = mybir.dt.bfloat16
    EXP = mybir.ActivationFunctionType.Exp
    IDENT = mybir.ActivationFunctionType.Identity
    COPY = mybir.ActivationFunctionType.Copy

    nc = bacc.Bacc(trn_type="TRN2", target_bir_lowering=False, debug=False)

    xT = nc.dram_tensor("xT", [NBAND, 128, NCH, BAND], bf16, kind="ExternalInput").ap()
    wq = nc.dram_tensor("wq", [GROUP, 128, NCH, 128], bf16, kind="ExternalInput").ap()
    wk = nc.dram_tensor("wk", [128, NCH, 128], bf16, kind="ExternalInput").ap()
    wv = nc.dram_tensor("wv", [128, NCH, 128], bf16, kind="ExternalInput").ap()
    wo = nc.dram_tensor("wo", [GROUP, 128, HIDDEN], bf16, kind="ExternalInput").ap()
    bq = nc.dram_tensor("bq", [128, GROUP], f32, kind="ExternalInput").ap()
    onesk = nc.dram_tensor("onesk", [128, 128], bf16, kind="ExternalInput").ap()
    out = nc.dram_tensor("out", [S, HIDDEN], f32, kind="ExternalOutput").ap()

    with tile.TileContext(nc) as tc:
        with (
            tc.tile_pool(name="const", bufs=1) as constp,
            tc.tile_pool(name="wts", bufs=1) as wtsp,
            tc.tile_pool(name="xb", bufs=2) as xbp,
            tc.tile_pool(name="qkv", bufs=1) as qkvp,
            tc.tile_pool(name="ptbuf", bufs=2) as ptp,
            tc.tile_pool(name="rbuf", bufs=2) as rp,
            tc.tile_pool(name="dens", bufs=2) as densp,
            tc.tile_pool(name="ytbuf", bufs=8) as ytp,
            tc.tile_pool(name="outbuf", bufs=2) as outp,
        ):
            # ---- DMAs in consumption order ----
            onesk_t = constp.tile([128, 128], bf16, name="onesk_t")
            nc.sync.dma_start(out=onesk_t[:, :], in_=onesk)
            bq_t = constp.tile([128, GROUP], f32, name="bq_t")
            nc.sync.dma_start(out=bq_t[:, :], in_=bq)
            ident = constp.tile([128, 128], f32, name="ident")
            make_identity(nc, ident[:, :])

            wk_t = wtsp.tile([128, NCH, 128], bf16, name="wk_t")
            nc.sync.dma_start(out=wk_t[:, :, :], in_=wk)

            # band 0 issued right after wk; remaining weights before band 1
            bands = [None] * NBAND
            bands[0] = xbp.tile([128, NCH, BAND], bf16, name="band", tag="band")
            nc.sync.dma_start(out=bands[0][:, :, :], in_=xT[0])

            wv_t = wtsp.tile([128, NCH, 128], bf16, name="wv_t")
            nc.sync.dma_start(out=wv_t[:, :, :], in_=wv)
            wq_t = []
            for g in range(GROUP):
                t = wtsp.tile([128, NCH, 128], bf16, name=f"wq_t{g}", tag=f"wq{g}")
                nc.sync.dma_start(out=t[:, :, :], in_=wq[g])
                wq_t.append(t)
            bands[1] = xbp.tile([128, NCH, BAND], bf16, name="band", tag="band")
            nc.sync.dma_start(out=bands[1][:, :, :], in_=xT[1])
            wo_t = []
            for g in range(GROUP):
                t = wtsp.tile([128, HIDDEN], bf16, name=f"wo_t{g}", tag=f"wo{g}")
                nc.sync.dma_start(out=t[:, :], in_=wo[g])
                wo_t.append(t)

            # ---- persistent activations ----
            qt_t = []
            for g in range(GROUP):
                t = qkvp.tile([128, S], bf16, name=f"qt{g}", tag=f"qt{g}")
                qt_t.append(t)
            kt_t = qkvp.tile([128, S], bf16, name="kt_t")
            v_t = qkvp.tile([128, S], bf16, name="v_t")
            vt_f = qkvp.tile([128, S], f32, name="vt_f")

            # =============== phase 1: projections ===============
            with tc.tile_pool(name="psA", bufs=1, space="PSUM") as psA:
                for bd in range(NBAND):
                    if bands[bd] is None:
                        bands[bd] = xbp.tile(
                            [128, NCH, BAND], bf16, name="band", tag="band"
                        )
                        nc.sync.dma_start(out=bands[bd][:, :, :], in_=xT[bd])
                    band = bands[bd]
                    bsl = slice(bd * BAND, (bd + 1) * BAND)

                    # K^T accumulation
                    pk = psA.tile([128, BAND], f32, name="pk", tag="pacc", bufs=3)
                    for c in range(NCH):
                        nc.tensor.matmul(
                            out=pk[:, :],
                            lhsT=wk_t[:, c, :],
                            rhs=band[:, c, :],
                            start=(c == 0), stop=(c == NCH - 1),
                        )
                    nc.scalar.activation(kt_t[:, bsl], pk[:, :], COPY)

                    # V^T accumulation (f32, transposed to V per 128-block later)
                    pv = psA.tile([128, BAND], f32, name="pv", tag="pacc", bufs=3)
                    for c in range(NCH):
                        nc.tensor.matmul(
                            out=pv[:, :],
                            lhsT=wv_t[:, c, :],
                            rhs=band[:, c, :],
                            start=(c == 0), stop=(c == NCH - 1),
                        )
                    nc.scalar.activation(vt_f[:, bsl], pv[:, :], COPY)

                    # Q^T per local head
                    for g in range(GROUP):
                        pq = psA.tile([128, BAND], f32, name="pq", tag="pacc", bufs=3)
                        for c in range(NCH):
                            nc.tensor.matmul(
                                out=pq[:, :],
                                lhsT=wq_t[g][:, c, :],
                                rhs=band[:, c, :],
                                start=(c == 0), stop=(c == NCH - 1),
                            )
                        nc.scalar.activation(
                            qt_t[g][:, bsl], pq[:, :], IDENT,
                            bias=bq_t[:, g:g + 1],
                        )

                    # transpose V^T band -> V (BAND//128 sk-tiles per band)
                    for t in range(BAND // 128):
                        sk = bd * (BAND // 128) + t
                        pt = psA.tile([128, 128], f32, name="ptr", tag="pacc", bufs=3)
                        nc.tensor.transpose(
                            pt[:, :], vt_f[:, sk * 128:(sk + 1) * 128], ident[:, :]
                        )
                        nc.scalar.activation(
                            v_t[:, sk * 128:(sk + 1) * 128], pt[:, :], COPY
                        )

            # =============== phase 2+3: attention + out-projection ===============
            with tc.tile_pool(name="psB", bufs=1, space="PSUM") as psB:
                yt_all = {}
                pending = [None]  # deferred normalize closure

                def flush_pending():
                    if pending[0] is not None:
                        pending[0]()
                        pending[0] = None

                def attn_group(qt, g):
                    qsl = slice(qt * QTILE, (qt + 1) * QTILE)
                    py = psB.tile([128, QTILE], f32, name="py", tag="yacc", bufs=2)
                    pt_all = ptp.tile(
                        [128, NSKP, 2 * QTILE], bf16, name="pt", tag="pt"
                    )
                    # DVE accumulation chain in bf16 (2x DVE rate; positive
                    # sums, so the rounding error averages out in den)
                    R = rp.tile([128, 2 * QTILE], bf16, name="racc", tag="racc")
                    for skp in range(NSKP):
                        ps = psB.tile(
                            [128, 2 * QTILE], f32, name="ps", tag="sc", bufs=2
                        )
                        for half in range(2):
                            sk = 2 * skp + half
                            nc.tensor.matmul(
                                out=ps[:, half * QTILE:(half + 1) * QTILE],
                                lhsT=kt_t[:, sk * 128:(sk + 1) * 128],
                                rhs=qt_t[g][:, qsl],
                                start=True, stop=True,
                            )
                        nc.scalar.activation(
                            pt_all[:, skp, :], ps[:, :], EXP, scale=SCALE
                        )
                        if skp == 1:
                            nc.vector.tensor_add(
                                R[:, :], pt_all[:, 0, :], pt_all[:, 1, :]
                            )
                        elif skp >= 2:
                            nc.vector.tensor_add(
                                R[:, :], R[:, :], pt_all[:, skp, :]
                            )
                        if skp >= 1:
                            # AV for the previous sk pair (scores stay one
                            # pair ahead so exp latency is hidden)
                            pskp = skp - 1
                            for half in range(2):
                                sk = 2 * pskp + half
                                nc.tensor.matmul(
                                    out=py[:, :],
                                    lhsT=v_t[:, sk * 128:(sk + 1) * 128],
                                    rhs=pt_all[:, pskp,
                                               half * QTILE:(half + 1) * QTILE],
                                    start=(sk == 0), stop=False,
                                )
                        if skp == 1:
                            flush_pending()
                    for half in range(2):
                        sk = 2 * (NSKP - 1) + half
                        nc.tensor.matmul(
                            out=py[:, :],
                            lhsT=v_t[:, sk * 128:(sk + 1) * 128],
                            rhs=pt_all[:, NSKP - 1,
                                       half * QTILE:(half + 1) * QTILE],
                            start=False, stop=(sk == NSK - 1),
                        )

                    def normalize(qt=qt, g=g, py=py, R=R):
                        rh = rp.tile([128, QTILE], bf16, name="rh", tag="rh")
                        nc.vector.tensor_add(
                            rh[:, :], R[:, 0:QTILE], R[:, QTILE:2 * QTILE]
                        )
                        # all-ones stationary: out[i, q] = sum_p rh[p, q] —
                        # partition-reduces AND replicates den on all 128
                        # partitions in one 512-row matmul
                        pden = psB.tile(
                            [128, QTILE], f32, name="pden", tag="den", bufs=2
                        )
                        nc.tensor.matmul(
                            out=pden[:, :],
                            lhsT=onesk_t[:, :],
                            rhs=rh[:, :],
                            start=True, stop=True,
                        )
                        rb = densp.tile([128, QTILE], f32, name="rb", tag="rb")
                        nc.vector.reciprocal_approx_fast(rb[:, :], pden[:, :])
                        yt = ytp.tile([128, QTILE], bf16, name="yt", tag="yt")
                        nc.vector.tensor_mul(yt[:, :], py[:, :], rb[:, :])
                        yt_all[(qt, g)] = yt

                    pending[0] = normalize

                def outproj_block(qt, i):
                    outs = outp.tile([128, HIDDEN], f32, name="outs", tag="outs")
                    po = [
                        psB.tile([128, 2 * QTILE], f32, name=f"po{jp}",
                                 tag="sc", bufs=2)
                        for jp in range(2)
                    ]
                    for g in range(GROUP):
                        lhs = yt_all[(qt, g)][:, i * 128:(i + 1) * 128]
                        for j in range(4):
                            nc.tensor.matmul(
                                out=po[j // 2][:, (j % 2) * 512:(j % 2 + 1) * 512],
                                lhsT=lhs,
                                rhs=wo_t[g][:, j * 512:(j + 1) * 512],
                                start=(g == 0), stop=(g == GROUP - 1),
                            )
                    nc.scalar.activation(outs[:, 0:1024], po[0][:, :], COPY)
                    nc.vector.tensor_copy(outs[:, 1024:2048], po[1][:, :])
                    r0 = qt * QTILE + i * 128
                    nc.sync.dma_start(out=out[r0:r0 + 128, :], in_=outs[:, :])

                for qt in range(NQT):
                    for g in range(GROUP):
                        attn_group(qt, g)
                        if qt >= 1:
                            outproj_block(qt - 1, g)
                flush_pending()
                for i in range(QTILE // 128):
                    outproj_block(NQT - 1, i)

    nc.finalize()
    return nc


def _get_nc():
    if "nc" not in _CACHE:
        _CACHE["nc"] = _build()
    return _CACHE["nc"]


def kernel(x, Wq, bq, Wk, bk, Wv, bv, Wo, bo):
    global LAST_RESULTS
    from concourse.bass_utils import run_bass_kernel_spmd

    bf = ml_dtypes.bfloat16
    x = np.asarray(x, np.float32)
    Wq = np.asarray(Wq, np.float32)
    Wk = np.asarray(Wk, np.float32)
    Wv = np.asarray(Wv, np.float32)
    Wo = np.asarray(Wo, np.float32)
    bq = np.asarray(bq, np.float32)
    bv = np.asarray(bv, np.float32)
    bo = np.asarray(bo, np.float32)

    nc = _get_nc()

    onesk_np = np.ones((128, 128), bf)

    in_maps = []
    for c in range(8):
        b, h = divmod(c, NKV)
        xTb = x[b].T  # [HIDDEN, S]
        xTh = np.ascontiguousarray(
            xTb.reshape(NCH, 128, NBAND, BAND).transpose(2, 1, 0, 3)
        ).astype(bf)
        # wq[g]: [128, NCH, 128] per local head
        wqh = np.ascontiguousarray(
            Wq[:, h * 512:(h + 1) * 512]
            .reshape(NCH, 128, GROUP, 128).transpose(2, 1, 0, 3)
        ).astype(bf)
        wkh = np.ascontiguousarray(
            Wk[:, h * 128:(h + 1) * 128].reshape(NCH, 128, 128).transpose(1, 0, 2)
        ).astype(bf)
        wvh = np.ascontiguousarray(
            Wv[:, h * 128:(h + 1) * 128].reshape(NCH, 128, 128).transpose(1, 0, 2)
        ).astype(bf)
        woh = np.ascontiguousarray(
            Wo[h * 512:(h + 1) * 512, :].reshape(GROUP, 128, HIDDEN)
        ).astype(bf)
        bqh = np.ascontiguousarray(
            bq[h * 512:(h + 1) * 512].reshape(GROUP, 128).T
        )
        in_maps.append({
            "xT": xTh, "wq": wqh, "wk": wkh, "wv": wvh, "wo": woh,
            "bq": bqh, "onesk": onesk_np,
        })

    res = run_bass_kernel_spmd(
        nc, in_maps, list(range(8)), trace=TRACE, tmpdir=TMPDIR
    )
    LAST_RESULTS = res

    # host-side constant bias: (bv repeated per head group) @ Wo + bo
    bv_rep = np.broadcast_to(
        bv.reshape(NKV, 1, D), (NKV, GROUP, D)
    ).reshape(HIDDEN)
    bias_row = bv_rep @ Wo + bo  # [HIDDEN]

    out = np.empty((B, S, HIDDEN), np.float32)
    for b in range(B):
        acc = res.results[b * NKV + 0]["out"].astype(np.float32)
        for h in range(1, NKV):
            acc = acc + res.results[b * NKV + h]["out"]
        out[b] = acc + bias_row
    return out


# revision 15
# speedup vs baseline: 1.3371x; 1.0569x over previous
"""GQA attention kernel for 8 Trainium2 NeuronCores — v2.

Sharding: core c = 4*b + h handles batch b (of 2) and kv-head h (of 4),
i.e. one kv head + its 4 grouped query heads. Each core computes its head
group's partial contribution to the output projection; the host sums the
4 partials per batch. No collectives.

v2 changes vs v1 (461us):
- all matmul inputs bf16 (x, Wq/k/v already Wo) — halves DMA traffic;
  bf16 matmul is 1 cyc/row like f32r, but ldweights are cheaper
- softmax denominator: accumulate the exp tiles into R via two parallel
  chains (GpSimd: even sk-pairs, DVE: odd sk-pairs), fold, then ONE
  512-row matmul with an all-ones [128,128] stationary per (qtile, head)
  — the PE both reduces over partitions AND replicates den across all
  128 output partitions, so no gpsimd broadcast is needed. Replaces the
  per-sk ones-matmuls that were 1/3 of attention-phase PE rows.
- reciprocal_approx_fast instead of reciprocal (~5x faster)
- software pipelining: outproj(qt-1) i-blocks interleaved between
  attn(qt) head-groups — pure-PE outproj stretches let the scalar
  engine (exp is 1.11us/sk-pair vs PE's 0.85us) catch up; scores
  emitted one sk-pair ahead of AV; normalize deferred one head-group
- BAND 256->512: ap-512 projection matmuls

Device math per core (S=2048, H=2048, d=128):
  QT_g = (x @ Wq_g + bq_g)^T          [d, S]   g=0..3   (bf16)
  KT   = (x @ Wk_h)^T                 [d, S]            (bk cancels in softmax)
  V    = x @ Wv_h                     [S, d]   (computed as V^T then PE-transposed)
  S^T  = KT^T-blocks @ QT             [Sk, Sq]
  P^T  = exp(SCALE * S^T)             (bf16, no max-subtraction: |s| <~ 6)
  den  = ones.T @ (DVE-sum of P^T tiles)
  y^T  = V^T-blocks.T @ P^T (PSUM accum);  yT := y^T * (1/den)
  out += yT_g^T @ Wo_g                [S, H]  (partial over this head group)
Host: out[b] = sum_h partial + (bv_rep @ Wo + bo).
"""

import numpy as np
import ml_dtypes

B = 2
S = 2048
HIDDEN = 2048
NKV = 4
GROUP = 4
D = 128
SCALE = D ** -0.5

BAND = 512            # S-columns per projection band
NBAND = S // BAND     # 4
NCH = HIDDEN // 128   # 16 contraction chunks
QTILE = 512           # queries per attention tile
NQT = S // QTILE      # 4
NSK = S // 128        # 16 key tiles
NSKP = NSK // 2       # 8 sk pairs

_CACHE = {}
LAST_RESULTS = None
TRACE = False
TMPDIR = None


def _build():
    import concourse.bass as bass
    import concourse.bacc as bacc
    import concourse.mybir as mybir
    import concourse.tile as tile
    from concourse.masks import make_identity

    f32 = mybir.dt.float32
    bf16 = mybir.dt.bfloat16
    EXP = mybir.ActivationFunctionType.Exp
    IDENT = mybir.ActivationFunctionType.Identity
    COPY = mybir.ActivationFunctionType.Copy

    nc = bacc.Bacc(trn_type="TRN2", target_bir_lowering=False, debug=False)

    xT = nc.dram_tensor("xT", [NBAND, 128, NCH, BAND], bf16, kind="ExternalInput").ap()
    wq = nc.dram_tensor("wq", [GROUP, 128, NCH, 128], bf16, kind="ExternalInput").ap()
    wk = nc.dram_tensor("wk", [128, NCH, 128], bf16, kind="ExternalInput").ap()
    wv = nc.dram_tensor("wv", [128, NCH, 128], bf16, kind="ExternalInput").ap()
    wo = nc.dram_tensor("wo", [GROUP, 128, HIDDEN], bf16, kind="ExternalInput").ap()
    bq = nc.dram_tensor("bq", [128, GROUP], f32, kind="ExternalInput").ap()
    onesk = nc.dram_tensor("onesk", [128, 128], bf16, kind="ExternalInput").ap()
    out = nc.dram_tensor("out", [S, HIDDEN], f32, kind="ExternalOutput").ap()

    with tile.TileContext(nc) as tc:
        with (
            tc.tile_pool(name="const", bufs=1) as constp,
            tc.tile_pool(name="wts", bufs=1) as wtsp,
            tc.tile_pool(name="xb", bufs=2) as xbp,
            tc.tile_pool(name="qkv", bufs=1) as qkvp,
            tc.tile_pool(name="ptbuf", bufs=2) as ptp,
            tc.tile_pool(name="rbuf", bufs=2) as rp,
            tc.tile_pool(name="dens", bufs=2) as densp,
            tc.tile_pool(name="ytbuf", bufs=8) as ytp,
            tc.tile_pool(name="outbuf", bufs=2) as outp,
        ):
            # ---- DMAs in consumption order ----
            onesk_t = constp.tile([128, 128], bf16, name="onesk_t")
            nc.sync.dma_start(out=onesk_t[:, :], in_=onesk)
            bq_t = constp.tile([128, GROUP], f32, name="bq_t")
            nc.sync.dma_start(out=bq_t[:, :], in_=bq)
            ident = constp.tile([128, 128], f32, name="ident")
            make_identity(nc, ident[:, :])

            wk_t = wtsp.tile([128, NCH, 128], bf16, name="wk_t")
            nc.sync.dma_start(out=wk_t[:, :, :], in_=wk)

            # band 0 issued right after wk; remaining weights before band 1
            bands = [None] * NBAND
            bands[0] = xbp.tile([128, NCH, BAND], bf16, name="band", tag="band")
            nc.sync.dma_start(out=bands[0][:, :, :], in_=xT[0])

            wv_t = wtsp.tile([128, NCH, 128], bf16, name="wv_t")
            nc.sync.dma_start(out=wv_t[:, :, :], in_=wv)
            wq_t = []
            for g in range(GROUP):
                t = wtsp.tile([128, NCH, 128], bf16, name=f"wq_t{g}", tag=f"wq{g}")
                nc.sync.dma_start(out=t[:, :, :], in_=wq[g])
                wq_t.append(t)
            bands[1] = xbp.tile([128, NCH, BAND], bf16, name="band", tag="band")
            nc.sync.dma_start(out=bands[1][:, :, :], in_=xT[1])
            wo_t = []
            for g in range(GROUP):
                t = wtsp.tile([128, HIDDEN], bf16, name=f"wo_t{g}", tag=f"wo{g}")
                nc.sync.dma_start(out=t[:, :], in_=wo[g])
                wo_t.append(t)

            # ---- persistent activations ----
            qt_t = []
            for g in range(GROUP):
                t = qkvp.tile([128, S], bf16, name=f"qt{g}", tag=f"qt{g}")
                qt_t.append(t)
            kt_t = qkvp.tile([128, S], bf16, name="kt_t")
            v_t = qkvp.tile([128, S], bf16, name="v_t")
            vt_f = qkvp.tile([128, S], f32, name="vt_f")

            # =============== phase 1: projections ===============
            with tc.tile_pool(name="psA", bufs=1, space="PSUM") as psA:
                for bd in range(NBAND):
                    if bands[bd] is None:
                        bands[bd] = xbp.tile(
                            [128, NCH, BAND], bf16, name="band", tag="band"
                        )
                        nc.sync.dma_start(out=bands[bd][:, :, :], in_=xT[bd])
                    band = bands[bd]
                    bsl = slice(bd * BAND, (bd + 1) * BAND)

                    # K^T accumulation
                    pk = psA.tile([128, BAND], f32, name="pk", tag="pacc", bufs=3)
                    for c in range(NCH):
                        nc.tensor.matmul(
                            out=pk[:, :],
                            lhsT=wk_t[:, c, :],
                            rhs=band[:, c, :],
                            start=(c == 0), stop=(c == NCH - 1),
                        )
                    nc.scalar.activation(kt_t[:, bsl], pk[:, :], COPY)

                    # V^T accumulation (f32, transposed to V per 128-block later)
                    pv = psA.tile([128, BAND], f32, name="pv", tag="pacc", bufs=3)
                    for c in range(NCH):
                        nc.tensor.matmul(
                            out=pv[:, :],
                            lhsT=wv_t[:, c, :],
                            rhs=band[:, c, :],
                            start=(c == 0), stop=(c == NCH - 1),
                        )
                    nc.scalar.activation(vt_f[:, bsl], pv[:, :], COPY)

                    # Q^T per local head
                    for g in range(GROUP):
                        pq = psA.tile([128, BAND], f32, name="pq", tag="pacc", bufs=3)
                        for c in range(NCH):
                            nc.tensor.matmul(
                                out=pq[:, :],
                                lhsT=wq_t[g][:, c, :],
                                rhs=band[:, c, :],
                                start=(c == 0), stop=(c == NCH - 1),
                            )
                        nc.scalar.activation(
                            qt_t[g][:, bsl], pq[:, :], IDENT,
                            bias=bq_t[:, g:g + 1],
                        )

                    # transpose V^T band -> V (BAND//128 sk-tiles per band)
                    for t in range(BAND // 128):
                        sk = bd * (BAND // 128) + t
                        pt = psA.tile([128, 128], f32, name="ptr", tag="pacc", bufs=3)
                        nc.tensor.transpose(
                            pt[:, :], vt_f[:, sk * 128:(sk + 1) * 128], ident[:, :]
                        )
                        nc.scalar.activation(
                            v_t[:, sk * 128:(sk + 1) * 128], pt[:, :], COPY
                        )

            # =============== phase 2+3: attention + out-projection ===============
            with tc.tile_pool(name="psB", bufs=1, space="PSUM") as psB:
                yt_all = {}
                pending = [None]  # deferred normalize closure

                def flush_pending():
                    if pending[0] is not None:
                        pending[0]()
                        pending[0] = None

                def attn_group(qt, g, proj_qt=None, proj_i=None):
                    """One head-group of attention over qtile qt. When
                    proj_qt/proj_i are given, the 16 out-projection matmuls
                    for block (proj_qt, proj_i) are interleaved one per sk
                    iteration, filling the PE while it would otherwise wait
                    on the scalar engine's exp (0.59us/sk vs 0.43us of
                    attention-only PE work per sk)."""
                    qsl = slice(qt * QTILE, (qt + 1) * QTILE)
                    py = psB.tile([128, QTILE], f32, name="py", tag="yacc", bufs=2)
                    pt_all = ptp.tile(
                        [128, NSK, QTILE], bf16, name="pt", tag="pt"
                    )
                    # DVE accumulation chain in bf16 (2x DVE rate; positive
                    # sums, so the rounding error averages out in den)
                    R = rp.tile([128, QTILE], bf16, name="racc", tag="racc")
                    outs = po = None
                    if proj_qt is not None:
                        outs = outp.tile([128, HIDDEN], f32, name="outs", tag="outs")

                    for sk in range(NSK):
                        ps = psB.tile([128, QTILE], f32, name="ps", tag="sc", bufs=3)
                        nc.tensor.matmul(
                            out=ps[:, :],
                            lhsT=kt_t[:, sk * 128:(sk + 1) * 128],
                            rhs=qt_t[g][:, qsl],
                            start=True, stop=True,
                        )
                        nc.scalar.activation(
                            pt_all[:, sk, :], ps[:, :], EXP, scale=SCALE
                        )
                        if sk == 1:
                            nc.vector.tensor_add(
                                R[:, :], pt_all[:, 0, :], pt_all[:, 1, :]
                            )
                        elif sk >= 2:
                            nc.vector.tensor_add(
                                R[:, :], R[:, :], pt_all[:, sk, :]
                            )
                        if sk >= 2:
                            # AV trails scores by two sk so exp latency and
                            # jitter stay hidden
                            psk = sk - 2
                            nc.tensor.matmul(
                                out=py[:, :],
                                lhsT=v_t[:, psk * 128:(psk + 1) * 128],
                                rhs=pt_all[:, psk, :],
                                start=(psk == 0), stop=False,
                            )
                        if proj_qt is not None:
                            # out-projection: po half a (cols 0:1024) over
                            # sk 0-7, half b (cols 1024:2048) over sk 8-15;
                            # each half accumulates j two column-512 groups
                            # over the 4 heads
                            if sk % 8 == 0:
                                po = psB.tile(
                                    [128, 2 * QTILE], f32, name="po",
                                    tag="po", bufs=1,
                                )
                            jh = sk % 8   # (g', j) pair index within half
                            gp, j = jh // 2, 2 * (sk // 8) + jh % 2
                            nc.tensor.matmul(
                                out=po[:, (j % 2) * 512:(j % 2 + 1) * 512],
                                lhsT=yt_all[(proj_qt, gp)][
                                    :, proj_i * 128:(proj_i + 1) * 128],
                                rhs=wo_t[gp][:, j * 512:(j + 1) * 512],
                                start=(gp == 0), stop=(gp == GROUP - 1),
                            )
                            if sk % 8 == 7:
                                half = sk // 8
                                csl = slice(half * 1024, half * 1024 + 1024)
                                nc.vector.tensor_copy(outs[:, csl], po[:, :])
                        if sk == 2:
                            flush_pending()
                    for psk in (NSK - 2, NSK - 1):
                        nc.tensor.matmul(
                            out=py[:, :],
                            lhsT=v_t[:, psk * 128:(psk + 1) * 128],
                            rhs=pt_all[:, psk, :],
                            start=False, stop=(psk == NSK - 1),
                        )
                    if proj_qt is not None:
                        r0 = proj_qt * QTILE + proj_i * 128
                        nc.sync.dma_start(out=out[r0:r0 + 128, :], in_=outs[:, :])

                    def normalize(qt=qt, g=g, py=py, R=R):
                        # all-ones stationary: out[i, q] = sum_p R[p, q] —
                        # partition-reduces AND replicates den on all 128
                        # partitions in one 512-row matmul
                        pden = psB.tile(
                            [128, QTILE], f32, name="pden", tag="den", bufs=1
                        )
                        nc.tensor.matmul(
                            out=pden[:, :],
                            lhsT=onesk_t[:, :],
                            rhs=R[:, :],
                            start=True, stop=True,
                        )
                        rb = densp.tile([128, QTILE], f32, name="rb", tag="rb")
                        nc.vector.reciprocal_approx_fast(rb[:, :], pden[:, :])
                        yt = ytp.tile([128, QTILE], bf16, name="yt", tag="yt")
                        nc.vector.tensor_mul(yt[:, :], py[:, :], rb[:, :])
                        yt_all[(qt, g)] = yt

                    pending[0] = normalize

                def outproj_block(qt, i):
                    outs = outp.tile([128, HIDDEN], f32, name="outs", tag="outs")
                    for half in range(2):
                        po = psB.tile(
                            [128, 2 * QTILE], f32, name="po", tag="po", bufs=1
                        )
                        for gp in range(GROUP):
                            lhs = yt_all[(qt, gp)][:, i * 128:(i + 1) * 128]
                            for jj in range(2):
                                j = 2 * half + jj
                                nc.tensor.matmul(
                                    out=po[:, jj * 512:(jj + 1) * 512],
                                    lhsT=lhs,
                                    rhs=wo_t[gp][:, j * 512:(j + 1) * 512],
                                    start=(gp == 0), stop=(gp == GROUP - 1),
                                )
                        csl = slice(half * 1024, half * 1024 + 1024)
                        nc.vector.tensor_copy(outs[:, csl], po[:, :])
                    r0 = qt * QTILE + i * 128
                    nc.sync.dma_start(out=out[r0:r0 + 128, :], in_=outs[:, :])

                for qt in range(NQT):
                    for g in range(GROUP):
                        if qt >= 1:
                            attn_group(qt, g, proj_qt=qt - 1, proj_i=g)
                        else:
                            attn_group(qt, g)
                flush_pending()
                for i in range(QTILE // 128):
                    outproj_block(NQT - 1, i)

    nc.finalize()
    return nc


def _get_nc():
    if "nc" not in _CACHE:
        _CACHE["nc"] = _build()
    return _CACHE["nc"]


def kernel(x, Wq, bq, Wk, bk, Wv, bv, Wo, bo):
    global LAST_RESULTS
    from concourse.bass_utils import run_bass_kernel_spmd

    bf = ml_dtypes.bfloat16
    x = np.asarray(x, np.float32)
    Wq = np.asarray(Wq, np.float32)
    Wk = np.asarray(Wk, np.float32)
    Wv = np.asarray(Wv, np.float32)
    Wo = np.asarray(Wo, np.float32)
    bq = np.asarray(bq, np.float32)
    bv = np.asarray(bv, np.float32)
    bo = np.asarray(bo, np.float32)

    nc = _get_nc()

    onesk_np = np.ones((128, 128), bf)

    in_maps = []
    for c in range(8):
        b, h = divmod(c, NKV)
        xTb = x[b].T  # [HIDDEN, S]
        xTh = np.ascontiguousarray(
            xTb.reshape(NCH, 128, NBAND, BAND).transpose(2, 1, 0, 3)
        ).astype(bf)
        # wq[g]: [128, NCH, 128] per local head
        wqh = np.ascontiguousarray(
            Wq[:, h * 512:(h + 1) * 512]
            .reshape(NCH, 128, GROUP, 128).transpose(2, 1, 0, 3)
        ).astype(bf)
        wkh = np.ascontiguousarray(
            Wk[:, h * 128:(h + 1) * 128].reshape(NCH, 128, 128).transpose(1, 0, 2)
        ).astype(bf)
        wvh = np.ascontiguousarray(
            Wv[:, h * 128:(h + 1) * 128].reshape(NCH, 128, 128).transpose(1, 0, 2)
        ).astype(bf)
        woh = np.ascontiguousarray(
            Wo[h * 512:(h + 1) * 512, :].reshape(GROUP, 128, HIDDEN)
        ).astype(bf)
        bqh = np.ascontiguousarray(
            bq[h * 512:(h + 1) * 512].reshape(GROUP, 128).T
        )
        in_maps.append({
            "xT": xTh, "wq": wqh, "wk": wkh, "wv": wvh, "wo": woh,
            "bq": bqh, "onesk": onesk_np,
        })

    res = run_bass_kernel_spmd(
        nc, in_maps, list(range(8)), trace=TRACE, tmpdir=TMPDIR
    )
    LAST_RESULTS = res

    # host-side constant bias: (bv repeated per head group) @ Wo + bo
    bv_rep = np.broadcast_to(
        bv.reshape(NKV, 1, D), (NKV, GROUP, D)
    ).reshape(HIDDEN)
    bias_row = bv_rep @ Wo + bo  # [HIDDEN]

    out = np.empty((B, S, HIDDEN), np.float32)
    for b in range(B):
        acc = res.results[b * NKV + 0]["out"].astype(np.float32)
        for h in range(1, NKV):
            acc = acc + res.results[b * NKV + h]["out"]
        out[b] = acc + bias_row
    return out


# revision 18
# speedup vs baseline: 1.3977x; 1.0454x over previous
"""GQA attention kernel for 8 Trainium2 NeuronCores — v2.

Sharding: core c = 4*b + h handles batch b (of 2) and kv-head h (of 4),
i.e. one kv head + its 4 grouped query heads. Each core computes its head
group's partial contribution to the output projection; the host sums the
4 partials per batch. No collectives.

v2 changes vs v1 (461us):
- all matmul inputs bf16 (x, Wq/k/v already Wo) — halves DMA traffic;
  bf16 matmul is 1 cyc/row like f32r, but ldweights are cheaper
- softmax denominator: accumulate the exp tiles into R via two parallel
  chains (GpSimd: even sk-pairs, DVE: odd sk-pairs), fold, then ONE
  512-row matmul with an all-ones [128,128] stationary per (qtile, head)
  — the PE both reduces over partitions AND replicates den across all
  128 output partitions, so no gpsimd broadcast is needed. Replaces the
  per-sk ones-matmuls that were 1/3 of attention-phase PE rows.
- reciprocal_approx_fast instead of reciprocal (~5x faster)
- software pipelining: outproj(qt-1) i-blocks interleaved between
  attn(qt) head-groups — pure-PE outproj stretches let the scalar
  engine (exp is 1.11us/sk-pair vs PE's 0.85us) catch up; scores
  emitted one sk-pair ahead of AV; normalize deferred one head-group
- BAND 256->512: ap-512 projection matmuls

Device math per core (S=2048, H=2048, d=128):
  QT_g = (x @ Wq_g + bq_g)^T          [d, S]   g=0..3   (bf16)
  KT   = (x @ Wk_h)^T                 [d, S]            (bk cancels in softmax)
  V    = x @ Wv_h                     [S, d]   (computed as V^T then PE-transposed)
  S^T  = KT^T-blocks @ QT             [Sk, Sq]
  P^T  = exp(SCALE * S^T)             (bf16, no max-subtraction: |s| <~ 6)
  den  = ones.T @ (DVE-sum of P^T tiles)
  y^T  = V^T-blocks.T @ P^T (PSUM accum);  yT := y^T * (1/den)
  out += yT_g^T @ Wo_g                [S, H]  (partial over this head group)
Host: out[b] = sum_h partial + (bv_rep @ Wo + bo).
"""

import numpy as np
import ml_dtypes

B = 2
S = 2048
HIDDEN = 2048
NKV = 4
GROUP = 4
D = 128
SCALE = D ** -0.5

BAND = 512            # S-columns per projection band
NBAND = S // BAND     # 4
NCH = HIDDEN // 128   # 16 contraction chunks
QTILE = 512           # queries per attention tile
NQT = S // QTILE      # 4
NSK = S // 128        # 16 key tiles
NSKP = NSK // 2       # 8 sk pairs

_CACHE = {}
LAST_RESULTS = None
TRACE = False
TMPDIR = None


def _build():
    import concourse.bass as bass
    import concourse.bacc as bacc
    import concourse.mybir as mybir
    import concourse.tile as tile
    from concourse.masks import make_identity

    f32 = mybir.dt.float32
    bf16 = mybir.dt.bfloat16
    EXP = mybir.ActivationFunctionType.Exp
    IDENT = mybir.ActivationFunctionType.Identity
    COPY = mybir.ActivationFunctionType.Copy

    nc = bacc.Bacc(trn_type="TRN2", target_bir_lowering=False, debug=False)

    xT = nc.dram_tensor("xT", [NBAND, 128, NCH, BAND], bf16, kind="ExternalInput").ap()
    wq = nc.dram_tensor("wq", [GROUP, 128, NCH, 128], bf16, kind="ExternalInput").ap()
    wk = nc.dram_tensor("wk", [128, NCH, 128], bf16, kind="ExternalInput").ap()
    wv = nc.dram_tensor("wv", [128, NCH, 128], bf16, kind="ExternalInput").ap()
    wo = nc.dram_tensor("wo", [GROUP, 128, HIDDEN], bf16, kind="ExternalInput").ap()
    bq = nc.dram_tensor("bq", [128, GROUP], f32, kind="ExternalInput").ap()
    onesk = nc.dram_tensor("onesk", [128, 128], bf16, kind="ExternalInput").ap()
    out = nc.dram_tensor("out", [S, HIDDEN], f32, kind="ExternalOutput").ap()

    with tile.TileContext(nc) as tc:
        with (
            tc.tile_pool(name="const", bufs=1) as constp,
            tc.tile_pool(name="wts", bufs=1) as wtsp,
            tc.tile_pool(name="xb", bufs=2) as xbp,
            tc.tile_pool(name="qkv", bufs=1) as qkvp,
            tc.tile_pool(name="ptbuf", bufs=2) as ptp,
            tc.tile_pool(name="rbuf", bufs=2) as rp,
            tc.tile_pool(name="dens", bufs=2) as densp,
            tc.tile_pool(name="ytbuf", bufs=8) as ytp,
            tc.tile_pool(name="outbuf", bufs=2) as outp,
        ):
            # ---- DMAs in consumption order ----
            onesk_t = constp.tile([128, 128], bf16, name="onesk_t")
            nc.sync.dma_start(out=onesk_t[:, :], in_=onesk)
            bq_t = constp.tile([128, GROUP], f32, name="bq_t")
            nc.sync.dma_start(out=bq_t[:, :], in_=bq)
            ident = constp.tile([128, 128], f32, name="ident")
            make_identity(nc, ident[:, :])

            wk_t = wtsp.tile([128, NCH, 128], bf16, name="wk_t")
            nc.sync.dma_start(out=wk_t[:, :, :], in_=wk)

            # band 0 split into 4 quarter-DMAs (separate tiles) so the first
            # K-projection chunk matmuls start as soon as the first quarter
            # lands instead of waiting for the whole 2.1 MB band
            bands = [None] * NBAND
            b0q = []
            for c4 in range(4):
                t = xbp.tile([128, 4, BAND], bf16, name=f"b0q{c4}", tag="band0q",
                             bufs=4)
                nc.sync.dma_start(out=t[:, :, :], in_=xT[0, :, 4 * c4:4 * c4 + 4, :])
                b0q.append(t)

            wv_t = wtsp.tile([128, NCH, 128], bf16, name="wv_t")
            nc.sync.dma_start(out=wv_t[:, :, :], in_=wv)
            wq_t = []
            for g in range(GROUP):
                t = wtsp.tile([128, NCH, 128], bf16, name=f"wq_t{g}", tag=f"wq{g}")
                nc.sync.dma_start(out=t[:, :, :], in_=wq[g])
                wq_t.append(t)
            bands[1] = xbp.tile([128, NCH, BAND], bf16, name="band", tag="band")
            nc.sync.dma_start(out=bands[1][:, :, :], in_=xT[1])
            wo_t = []
            for g in range(GROUP):
                t = wtsp.tile([128, HIDDEN], bf16, name=f"wo_t{g}", tag=f"wo{g}")
                nc.sync.dma_start(out=t[:, :], in_=wo[g])
                wo_t.append(t)

            # ---- persistent activations ----
            qt_t = []
            for g in range(GROUP):
                t = qkvp.tile([128, S], bf16, name=f"qt{g}", tag=f"qt{g}")
                qt_t.append(t)
            kt_t = qkvp.tile([128, S], bf16, name="kt_t")
            v_t = qkvp.tile([128, S], bf16, name="v_t")
            vt_f = qkvp.tile([128, S], f32, name="vt_f")

            # =============== phase 1: projections ===============
            with tc.tile_pool(name="psA", bufs=1, space="PSUM") as psA:
                for bd in range(NBAND):
                    if bd > 0 and bands[bd] is None:
                        bands[bd] = xbp.tile(
                            [128, NCH, BAND], bf16, name="band", tag="band"
                        )
                        nc.sync.dma_start(out=bands[bd][:, :, :], in_=xT[bd])

                    def bch(c, bd=bd):
                        if bd == 0:
                            return b0q[c // 4][:, c % 4, :]
                        return bands[bd][:, c, :]

                    bsl = slice(bd * BAND, (bd + 1) * BAND)

                    # K^T accumulation
                    pk = psA.tile([128, BAND], f32, name="pk", tag="pacc", bufs=3)
                    for c in range(NCH):
                        nc.tensor.matmul(
                            out=pk[:, :],
                            lhsT=wk_t[:, c, :],
                            rhs=bch(c),
                            start=(c == 0), stop=(c == NCH - 1),
                        )
                    nc.scalar.activation(kt_t[:, bsl], pk[:, :], COPY)

                    # V^T accumulation (f32, transposed to V per 128-block later)
                    pv = psA.tile([128, BAND], f32, name="pv", tag="pacc", bufs=3)
                    for c in range(NCH):
                        nc.tensor.matmul(
                            out=pv[:, :],
                            lhsT=wv_t[:, c, :],
                            rhs=bch(c),
                            start=(c == 0), stop=(c == NCH - 1),
                        )
                    nc.scalar.activation(vt_f[:, bsl], pv[:, :], COPY)

                    # Q^T per local head
                    for g in range(GROUP):
                        pq = psA.tile([128, BAND], f32, name="pq", tag="pacc", bufs=3)
                        for c in range(NCH):
                            nc.tensor.matmul(
                                out=pq[:, :],
                                lhsT=wq_t[g][:, c, :],
                                rhs=bch(c),
                                start=(c == 0), stop=(c == NCH - 1),
                            )
                        nc.scalar.activation(
                            qt_t[g][:, bsl], pq[:, :], IDENT,
                            bias=bq_t[:, g:g + 1],
                        )

                    # transpose V^T band -> V (BAND//128 sk-tiles per band)
                    for t in range(BAND // 128):
                        sk = bd * (BAND // 128) + t
                        pt = psA.tile([128, 128], f32, name="ptr", tag="pacc", bufs=3)
                        nc.tensor.transpose(
                            pt[:, :], vt_f[:, sk * 128:(sk + 1) * 128], ident[:, :]
                        )
                        nc.scalar.activation(
                            v_t[:, sk * 128:(sk + 1) * 128], pt[:, :], COPY
                        )

            # =============== phase 2+3: attention + out-projection ===============
            with tc.tile_pool(name="psB", bufs=1, space="PSUM") as psB:
                yt_all = {}
                pending = [None]  # deferred normalize closure

                def flush_pending():
                    if pending[0] is not None:
                        pending[0]()
                        pending[0] = None

                def attn_group(qt, g, proj_qt=None, proj_i=None):
                    """One head-group of attention over qtile qt. When
                    proj_qt/proj_i are given, the 16 out-projection matmuls
                    for block (proj_qt, proj_i) are interleaved one per sk
                    iteration, filling the PE while it would otherwise wait
                    on the scalar engine's exp (0.59us/sk vs 0.43us of
                    attention-only PE work per sk)."""
                    qsl = slice(qt * QTILE, (qt + 1) * QTILE)
                    py = psB.tile([128, QTILE], f32, name="py", tag="yacc", bufs=2)
                    pt_all = ptp.tile(
                        [128, NSK, QTILE], bf16, name="pt", tag="pt"
                    )
                    # DVE accumulation chain in bf16 (2x DVE rate; positive
                    # sums, so the rounding error averages out in den)
                    R = rp.tile([128, QTILE], bf16, name="racc", tag="racc")
                    outs = po = None
                    if proj_qt is not None:
                        outs = outp.tile([128, HIDDEN], f32, name="outs", tag="outs")

                    for sk in range(NSK):
                        ps = psB.tile([128, QTILE], f32, name="ps", tag="sc", bufs=3)
                        nc.tensor.matmul(
                            out=ps[:, :],
                            lhsT=kt_t[:, sk * 128:(sk + 1) * 128],
                            rhs=qt_t[g][:, qsl],
                            start=True, stop=True,
                        )
                        nc.scalar.activation(
                            pt_all[:, sk, :], ps[:, :], EXP, scale=SCALE
                        )
                        if sk == 1:
                            nc.vector.tensor_add(
                                R[:, :], pt_all[:, 0, :], pt_all[:, 1, :]
                            )
                        elif sk >= 2:
                            nc.vector.tensor_add(
                                R[:, :], R[:, :], pt_all[:, sk, :]
                            )
                        if sk >= 2:
                            # AV trails scores by two sk so exp latency and
                            # jitter stay hidden
                            psk = sk - 2
                            nc.tensor.matmul(
                                out=py[:, :],
                                lhsT=v_t[:, psk * 128:(psk + 1) * 128],
                                rhs=pt_all[:, psk, :],
                                start=(psk == 0), stop=False,
                            )
                        if proj_qt is not None:
                            # out-projection: po half a (cols 0:1024) over
                            # sk 0-7, half b (cols 1024:2048) over sk 8-15;
                            # each half accumulates j two column-512 groups
                            # over the 4 heads
                            if sk % 8 == 0:
                                po = psB.tile(
                                    [128, 2 * QTILE], f32, name="po",
                                    tag="po", bufs=1,
                                )
                            jh = sk % 8   # (g', j) pair index within half
                            gp, j = jh // 2, 2 * (sk // 8) + jh % 2
                            nc.tensor.matmul(
                                out=po[:, (j % 2) * 512:(j % 2 + 1) * 512],
                                lhsT=yt_all[(proj_qt, gp)][
                                    :, proj_i * 128:(proj_i + 1) * 128],
                                rhs=wo_t[gp][:, j * 512:(j + 1) * 512],
                                start=(gp == 0), stop=(gp == GROUP - 1),
                            )
                            if sk % 8 == 7:
                                half = sk // 8
                                csl = slice(half * 1024, half * 1024 + 1024)
                                nc.vector.tensor_copy(outs[:, csl], po[:, :])
                        if sk == 2:
                            flush_pending()
                    for psk in (NSK - 2, NSK - 1):
                        nc.tensor.matmul(
                            out=py[:, :],
                            lhsT=v_t[:, psk * 128:(psk + 1) * 128],
                            rhs=pt_all[:, psk, :],
                            start=False, stop=(psk == NSK - 1),
                        )
                    if proj_qt is not None:
                        r0 = proj_qt * QTILE + proj_i * 128
                        nc.sync.dma_start(out=out[r0:r0 + 128, :], in_=outs[:, :])

                    def normalize(qt=qt, g=g, py=py, R=R):
                        # all-ones stationary: out[i, q] = sum_p R[p, q] —
                        # partition-reduces AND replicates den on all 128
                        # partitions in one 512-row matmul
                        pden = psB.tile(
                            [128, QTILE], f32, name="pden", tag="den", bufs=1
                        )
                        nc.tensor.matmul(
                            out=pden[:, :],
                            lhsT=onesk_t[:, :],
                            rhs=R[:, :],
                            start=True, stop=True,
                        )
                        rb = densp.tile([128, QTILE], f32, name="rb", tag="rb")
                        nc.vector.reciprocal_approx_fast(rb[:, :], pden[:, :])
                        yt = ytp.tile([128, QTILE], bf16, name="yt", tag="yt")
                        nc.vector.tensor_mul(yt[:, :], py[:, :], rb[:, :])
                        yt_all[(qt, g)] = yt

                    pending[0] = normalize

                def outproj_block_tail(qt, i):
                    # attention is done here, so the "sc" psum tag is free:
                    # rotate 512-col chunks through its 3 buffers so the PE
                    # never waits on the DVE psum->sbuf copies
                    outs = outp.tile([128, HIDDEN], f32, name="outs", tag="outs")
                    for j in range(4):
                        poc = psB.tile([128, QTILE], f32, name="poc",
                                       tag="sc", bufs=3)
                        for gp in range(GROUP):
                            nc.tensor.matmul(
                                out=poc[:, :],
                                lhsT=yt_all[(qt, gp)][:, i * 128:(i + 1) * 128],
                                rhs=wo_t[gp][:, j * 512:(j + 1) * 512],
                                start=(gp == 0), stop=(gp == GROUP - 1),
                            )
                        nc.vector.tensor_copy(
                            outs[:, j * 512:(j + 1) * 512], poc[:, :]
                        )
                    r0 = qt * QTILE + i * 128
                    nc.sync.dma_start(out=out[r0:r0 + 128, :], in_=outs[:, :])

                for qt in range(NQT):
                    for g in range(GROUP):
                        if qt >= 1:
                            attn_group(qt, g, proj_qt=qt - 1, proj_i=g)
                        else:
                            attn_group(qt, g)
                flush_pending()
                for i in range(QTILE // 128):
                    outproj_block_tail(NQT - 1, i)

    nc.finalize()
    return nc


def _get_nc():
    if "nc" not in _CACHE:
        _CACHE["nc"] = _build()
    return _CACHE["nc"]


def kernel(x, Wq, bq, Wk, bk, Wv, bv, Wo, bo):
    global LAST_RESULTS
    from concourse.bass_utils import run_bass_kernel_spmd

    bf = ml_dtypes.bfloat16
    x = np.asarray(x, np.float32)
    Wq = np.asarray(Wq, np.float32)
    Wk = np.asarray(Wk, np.float32)
    Wv = np.asarray(Wv, np.float32)
    Wo = np.asarray(Wo, np.float32)
    bq = np.asarray(bq, np.float32)
    bv = np.asarray(bv, np.float32)
    bo = np.asarray(bo, np.float32)

    nc = _get_nc()

    onesk_np = np.ones((128, 128), bf)

    in_maps = []
    for c in range(8):
        b, h = divmod(c, NKV)
        xTb = x[b].T  # [HIDDEN, S]
        xTh = np.ascontiguousarray(
            xTb.reshape(NCH, 128, NBAND, BAND).transpose(2, 1, 0, 3)
        ).astype(bf)
        # wq[g]: [128, NCH, 128] per local head
        wqh = np.ascontiguousarray(
            Wq[:, h * 512:(h + 1) * 512]
            .reshape(NCH, 128, GROUP, 128).transpose(2, 1, 0, 3)
        ).astype(bf)
        wkh = np.ascontiguousarray(
            Wk[:, h * 128:(h + 1) * 128].reshape(NCH, 128, 128).transpose(1, 0, 2)
        ).astype(bf)
        wvh = np.ascontiguousarray(
            Wv[:, h * 128:(h + 1) * 128].reshape(NCH, 128, 128).transpose(1, 0, 2)
        ).astype(bf)
        woh = np.ascontiguousarray(
            Wo[h * 512:(h + 1) * 512, :].reshape(GROUP, 128, HIDDEN)
        ).astype(bf)
        bqh = np.ascontiguousarray(
            bq[h * 512:(h + 1) * 512].reshape(GROUP, 128).T
        )
        in_maps.append({
            "xT": xTh, "wq": wqh, "wk": wkh, "wv": wvh, "wo": woh,
            "bq": bqh, "onesk": onesk_np,
        })

    res = run_bass_kernel_spmd(
        nc, in_maps, list(range(8)), trace=TRACE, tmpdir=TMPDIR
    )
    LAST_RESULTS = res

    # host-side constant bias: (bv repeated per head group) @ Wo + bo
    bv_rep = np.broadcast_to(
        bv.reshape(NKV, 1, D), (NKV, GROUP, D)
    ).reshape(HIDDEN)
    bias_row = bv_rep @ Wo + bo  # [HIDDEN]

    out = np.empty((B, S, HIDDEN), np.float32)
    for b in range(B):
        acc = res.results[b * NKV + 0]["out"].astype(np.float32)
        for h in range(1, NKV):
            acc = acc + res.results[b * NKV + h]["out"]
        out[b] = acc + bias_row
    return out
